# revision 1
# baseline (speedup 1.0000x reference)
"""Trainium2 Bass kernel for nn_CapsRoutingLayer (capsule dynamic routing).

Sharding: data-parallel over batch. 8 NeuronCores, 8 batch elements each.
Each core streams the full (host bf16-cast, pre-transposed) W once, builds
x_hat in SBUF (bf16) via tile-packed PE matmuls, and runs the 3 routing
iterations on-core (DVE/ACT elementwise + small PE folds). No collectives.

Layout notes:
 - On-chip x_hat free layout is (step, r, d, o) with o innermost, so every
   broadcasted DVE operand keeps a unit-stride innermost dim (2x mode).
 - SBUF partition p = 32c + 8*nhat + b encodes 16 n-parts x 8 batch.
 - s (iteration 0, uniform c) is accumulated on the TensorEngine during the
   x_hat build via extra accumulating matmuls (lhsT = x chunk, no mask).

Self-contained: hardcodes all shapes from the problem spec.
  x: (64, 2048, 8) f32;  W: (2048, 32, 16, 8) f32  ->  v: (64, 32, 16) f32
"""

import sys

sys.path.insert(0, "/opt/trn_rl_repo")

import numpy as np
import ml_dtypes

# ---- problem sizes (hardcoded) ----
B_FULL, N, O, D, I = 64, 2048, 32, 16, 8
NCORES = 8
B = B_FULL // NCORES  # 8 batch elements per core
DO = D * O  # 512, on-chip innermost layout is (d, o)
NBLK = N // 16  # 128 16-n chunks
NSTEP = N // 64  # 32 build steps (4 chunks / 16 quads each)
R = 4  # quads per chunk (row groups)
N_ROUTING = 3

_NC = None


def _emit(tc, dram, ablate=()):
    import concourse.bass as bass
    from concourse import mybir

    nc = tc.nc
    BF = mybir.dt.bfloat16
    F32 = mybir.dt.float32
    AX = mybir.AxisListType
    ALU = mybir.AluOpType
    ACTF = mybir.ActivationFunctionType

    wt_d, xt_d, xmask_d, rep8_d, pfold_d, out_d = (
        dram["wt"], dram["xt"], dram["xmask"], dram["rep8"], dram["pfold"],
        dram["out"],
    )

    from contextlib import ExitStack

    ctx = ExitStack()
    const = ctx.enter_context(tc.tile_pool(name="const", bufs=1))
    persist = ctx.enter_context(tc.tile_pool(name="persist", bufs=1))
    wpool = ctx.enter_context(tc.tile_pool(name="wpool", bufs=2))
    xbpool = ctx.enter_context(tc.tile_pool(name="xbpool", bufs=8))
    scratch = ctx.enter_context(tc.tile_pool(name="scratch", bufs=1))

    # ---- constants / inputs resident in SBUF ----
    xmask = const.tile([128, 32], BF)
    nc.sync.dma_start(out=xmask[:], in_=xmask_d[:])
    rep8 = const.tile([8, 128], BF)
    nc.sync.dma_start(out=rep8[:], in_=rep8_d[:])
    pfold = const.tile([128, 8], F32)
    nc.sync.dma_start(out=pfold[:], in_=pfold_d[:])
    xt = const.tile([128, NBLK, B], BF)
    nc.sync.dma_start(out=xt[:], in_=xt_d[:].rearrange("n p b -> p n b"))

    # ---- persistent big tensors ----
    xh = persist.tile([128, NSTEP, R, DO], BF)  # x_hat, 128KB/partition
    logits = persist.tile([128, NSTEP, R, O], F32)  # routing logits b
    cw = persist.tile([128, NSTEP, R, O], BF)  # coupling coefficients c
    nc.vector.memset(logits[:], 0.0)

    # ---- phase 1: build x_hat (+ s0 on PE) ----
    with (
        tc.tile_pool(name="buildps", bufs=2, space="PSUM") as bps,
        tc.tile_pool(name="s0ps_pool", bufs=1, space="PSUM") as s0pool,
    ):
        s0ps = s0pool.tile([8, DO], F32)
        for s in range(NSTEP):
            wstep = wpool.tile([128, 4, DO], BF, tag="w", name=f"w_{s}")
            nc.sync.dma_start(
                out=wstep[:],
                in_=wt_d[4 * s : 4 * s + 4].rearrange("c p f -> p c f"),
            )
            wts = []
            xbs = []
            for c in range(4):
                w_t = wstep[:, c, :]
                xb = xbpool.tile([128, 32], BF, tag="xb", name=f"xb_{s}_{c}")
                xsl = (
                    xt[:, 4 * s + c, :]
                    .unsqueeze(1)
                    .broadcast_to([128, 4, B])
                )
                nc.vector.tensor_mul(
                    xb[:].rearrange("p (n b) -> p n b", n=4),
                    xmask[:].rearrange("p (n b) -> p n b", n=4),
                    xsl,
                )
                wts.append(w_t)
                xbs.append(xb)
                # s0 (uniform-c weighted sum over all n of this chunk)
                nc.tensor.matmul(
                    s0ps[:],
                    xt[:, 4 * s + c, :],
                    w_t,
                    start=(s == 0 and c == 0),
                    stop=(s == NSTEP - 1 and c == 3),
                    skip_group_check=True,
                )
            for h in range(2):  # r-halves: 2 psum banks each
                ps = bps.tile([128, 2, DO], F32, tag="bps", name=f"ps_{s}_{h}")
                for c in range(4):
                    for rl in range(2):
                        r = 2 * h + rl
                        nc.tensor.matmul(
                            ps[32 * c : 32 * c + 32, rl, :],
                            xbs[c][32 * r : 32 * r + 32, :],
                            wts[c][32 * r : 32 * r + 32, :],
                            start=True,
                            stop=True,
                            tile_position=(32 * r, 32 * c),
                        )
                if h == 0:
                    nc.scalar.copy(xh[:, s, 0:2, :], ps[:, :, :])
                else:
                    nc.vector.tensor_copy(xh[:, s, 2:4, :], ps[:, :, :])

        # ---- routing phase ----
        SS = 2  # a-step superstep width
        subacc = [
            scratch.tile([128, R, DO], BF, tag=f"sub{k}", name=f"sub{k}")
            for k in range(4)
        ]
        sr = scratch.tile([128, DO], F32, tag="sr")
        vrep = scratch.tile([128, DO], BF, tag="vrep")
        vsb = scratch.tile([8, DO], BF, tag="vsb")
        y2 = scratch.tile([128, SS, R, DO], BF, tag="y2")
        f1 = scratch.tile([128, SS, R, 8, O], BF, tag="f1")
        f2 = scratch.tile([128, SS, R, 4, O], BF, tag="f2")
        f3 = scratch.tile([128, SS, R, 2, O], BF, tag="f3")
        f4 = scratch.tile([128, SS, R, O], BF, tag="f4")
        ysc = y2[:, 0, :, :]  # s-step mult scratch aliases y2
        zred = scratch.tile([128, NSTEP * R], F32, tag="zred")
        zrb = scratch.tile([128, NSTEP * R], BF, tag="zrb")
        # squash smalls
        ssb = scratch.tile([8, DO], F32, tag="ssb")
        ssq = scratch.tile([8, DO], F32, tag="ssq")
        sq1 = scratch.tile([8, 8, O], F32, tag="sq1")
        sq2 = scratch.tile([8, 4, O], F32, tag="sq2")
        sq3 = scratch.tile([8, 2, O], F32, tag="sq3")
        n2 = scratch.tile([8, O], F32, tag="n2")
        nr = scratch.tile([8, O], F32, tag="nr")
        den = scratch.tile([8, O], F32, tag="den")
        fac = scratch.tile([8, O], F32, tag="fac")
        vout = scratch.tile([8, DO], F32, tag="vout")

        def s_step():
            # s[b, do] = sum_n c[b,n,o] * xh[b,n,do]
            for s in range(NSTEP):
                csl = (
                    cw[:, s, :, :]
                    .unsqueeze(2)
                    .broadcast_to([128, R, D, O])
                )
                nc.vector.tensor_mul(
                    ysc.rearrange("p r (d o) -> p r d o", d=D),
                    xh[:, s, :, :].rearrange("p r (d o) -> p r d o", d=D),
                    csl,
                )
                k = s % 4
                if s < 4:
                    nc.vector.tensor_copy(subacc[k][:], ysc)
                else:
                    nc.vector.tensor_add(subacc[k][:], subacc[k][:], ysc)
            # combine sub-accumulators (bf16, in place)
            nc.vector.tensor_add(subacc[0][:], subacc[0][:], subacc[1][:])
            nc.vector.tensor_add(subacc[2][:], subacc[2][:], subacc[3][:])
            nc.vector.tensor_add(subacc[0][:], subacc[0][:], subacc[2][:])
            # fold over r (f32)
            nc.vector.tensor_add(sr[:], subacc[0][:, 0, :], subacc[0][:, 1, :])
            nc.vector.tensor_add(sr[:], sr[:], subacc[0][:, 2, :])
            nc.vector.tensor_add(sr[:], sr[:], subacc[0][:, 3, :])
            # partition fold: s[b, do] = sum_{p : p%8==b} sr[p, do]
            sps = rps.tile([8, DO], F32, tag="sps", name="sps")
            nc.tensor.matmul(sps[:], pfold[:], sr[:], start=True, stop=True)
            return sps

        def squash(sps, it, scale):
            last = it == N_ROUTING - 1
            nc.vector.tensor_copy(ssb[:], sps[:])
            nc.vector.tensor_mul(ssq[:], ssb[:], ssb[:])
            sv3 = ssq[:].rearrange("b (d o) -> b d o", d=D)
            nc.vector.tensor_add(sq1[:], sv3[:, 0:8, :], sv3[:, 8:16, :])
            nc.vector.tensor_add(sq2[:], sq1[:, 0:4, :], sq1[:, 4:8, :])
            nc.vector.tensor_add(sq3[:], sq2[:, 0:2, :], sq2[:, 2:4, :])
            nc.vector.tensor_add(n2[:], sq3[:, 0, :], sq3[:, 1, :])
            if scale != 1.0:
                nc.vector.tensor_scalar_mul(n2[:], n2[:], scale * scale)
            nc.scalar.activation(nr[:], n2[:], ACTF.Sqrt)
            nc.vector.tensor_scalar_add(den[:], n2[:], 1.0)
            nc.vector.reciprocal(fac[:], den[:])
            nc.vector.tensor_mul(fac[:], fac[:], nr[:])
            if scale != 1.0:
                nc.vector.tensor_scalar_mul(fac[:], fac[:], scale)
            fb = fac[:].unsqueeze(1).broadcast_to([8, D, O])
            sv = ssb[:].rearrange("b (d o) -> b d o", d=D)
            nc.vector.tensor_mul(vsb[:].rearrange("b (d o) -> b d o", d=D), sv, fb)
            if last:
                nc.vector.tensor_mul(
                    vout[:].rearrange("b (d o) -> b d o", d=D), sv, fb
                )
                nc.sync.dma_start(out=out_d[:], in_=vout[:])

        def a_step():
            # logits[b,n,o] += sum_d v[b,(d,o)] * xh[b,n,(d,o)]
            vps = rps.tile([128, DO], F32, tag="vps", name="vps")
            nc.tensor.matmul(vps[:], rep8[:], vsb[:], start=True, stop=True)
            nc.vector.tensor_copy(vrep[:], vps[:])
            for ss in range(NSTEP // SS):
                vb = (
                    vrep[:]
                    .unsqueeze(1)
                    .unsqueeze(1)
                    .broadcast_to([128, SS, R, DO])
                )
                nc.vector.tensor_mul(
                    y2[:], xh[:, SS * ss : SS * ss + SS, :, :], vb
                )
                yv = y2[:].rearrange("p a r (d o) -> p a r d o", d=D)
                nc.vector.tensor_add(f1[:], yv[:, :, :, 0:8, :], yv[:, :, :, 8:16, :])
                nc.vector.tensor_add(f2[:], f1[:, :, :, 0:4, :], f1[:, :, :, 4:8, :])
                nc.gpsimd.tensor_add(f3[:], f2[:, :, :, 0:2, :], f2[:, :, :, 2:4, :])
                nc.gpsimd.tensor_add(f4[:], f3[:, :, :, 0, :], f3[:, :, :, 1, :])
                lsl = logits[:, SS * ss : SS * ss + SS, :, :]
                nc.gpsimd.tensor_add(lsl, lsl, f4[:])

        def softmax():
            # cw = softmax(logits, axis=o); logits bounded, skip max-sub
            nc.scalar.activation(cw[:], logits[:], ACTF.Exp)
            nc.vector.tensor_reduce(
                zred[:],
                cw[:].rearrange("p s r o -> p (s r) o"),
                axis=AX.X,
                op=ALU.add,
            )
            nc.vector.reciprocal(zred[:], zred[:])
            nc.vector.tensor_copy(zrb[:], zred[:])
            zb = zrb[:].unsqueeze(2).broadcast_to([128, NSTEP * R, O])
            cv = cw[:].rearrange("p s r o -> p (s r) o")
            nc.vector.tensor_mul(cv, cv, zb)

        with tc.tile_pool(name="routps", bufs=1, space="PSUM") as rps:
            for it in range(N_ROUTING):
                if it == 0:
                    squash(s0ps, it, scale=1.0 / O)
                else:
                    if "softmax" not in ablate:
                        softmax()
                    sps = s_step() if "s" not in ablate else s0ps
                    squash(sps, it, scale=1.0)
                if it < N_ROUTING - 1 and "a" not in ablate:
                    a_step()

    ctx.close()


def build_nc(ablate=()):
    import concourse.bass as bass
    import concourse.tile as tile
    from concourse import bacc, mybir

    BF = mybir.dt.bfloat16
    F32 = mybir.dt.float32
    nc = bacc.Bacc(
        "TRN2",
        target_bir_lowering=False,
        debug=False,
        enable_asserts=False,
        num_devices=NCORES,
    )
    dram = {
        "wt": nc.dram_tensor("wt", [NBLK, 128, DO], BF, kind="ExternalInput").ap(),
        "xt": nc.dram_tensor("xt", [NBLK, 128, B], BF, kind="ExternalInput").ap(),
        "xmask": nc.dram_tensor("xmask", [128, 32], BF, kind="ExternalInput").ap(),
        "rep8": nc.dram_tensor("rep8", [8, 128], BF, kind="ExternalInput").ap(),
        "pfold": nc.dram_tensor("pfold", [128, 8], F32, kind="ExternalInput").ap(),
        "out": nc.dram_tensor("out", [B, DO], F32, kind="ExternalOutput").ap(),
    }
    with tile.TileContext(nc) as tc:
        _emit(tc, dram, ablate)
    nc.compile()
    return nc


def make_host_inputs(x, W):
    """Host-side layout prep. Returns per-core in_maps."""
    bf = ml_dtypes.bfloat16
    x = np.asarray(x, np.float32)
    W = np.asarray(W, np.float32)
    # W (N, O, D, I) -> (N, I, D, O) -> (NBLK, 128, DO)
    wt = (
        np.ascontiguousarray(W.transpose(0, 3, 2, 1))
        .reshape(NBLK, 128, DO)
        .astype(bf)
    )
    p = np.arange(128)
    f = np.arange(32)
    xmask = ((((p[:, None] % 32) // 8) == (f[None, :] // 8))).astype(bf)
    rep8 = (np.arange(8)[:, None] == (np.arange(128)[None, :] % 8)).astype(bf)
    pfold = ((np.arange(128)[:, None] % 8) == np.arange(8)[None, :]).astype(
        np.float32
    )
    in_maps = []
    for k in range(NCORES):
        xc = x[B * k : B * k + B]  # (B, N, I)
        xt = (
            np.ascontiguousarray(xc.transpose(1, 2, 0))
            .reshape(NBLK, 128, B)
            .astype(bf)
        )
        in_maps.append(
            {"wt": wt, "xt": xt, "xmask": xmask, "rep8": rep8, "pfold": pfold}
        )
    return in_maps


def assemble_out(core_outs):
    """core_outs[k]: (B, DO) f32 in (d, o) layout -> (64, O, D) f32."""
    outs = [
        np.asarray(o, np.float32).reshape(B, D, O).transpose(0, 2, 1)
        for o in core_outs
    ]
    return np.ascontiguousarray(np.concatenate(outs, axis=0))


def run(x, W, trace=False):
    """Build (cached), execute on 8 cores, return (out, exec_time_ns)."""
    global _NC
    from concourse.bass_utils import run_bass_kernel_spmd

    if _NC is None:
        _NC = build_nc()
    in_maps = make_host_inputs(x, W)
    res = run_bass_kernel_spmd(
        _NC, in_maps, core_ids=list(range(NCORES)), trace=trace
    )
    out = assemble_out([res.results[k]["out"] for k in range(NCORES)])
    return out, res.exec_time_ns


def kernel(x, W):
    import time

    for attempt in range(3):
        try:
            out, _ = run(x, W, trace=False)
            return out
        except Exception:
            if attempt == 2:
                raise
            time.sleep(2.0)


def bench_hw(x, W, iters=30):
    """Repeat-execute the kernel NEFF on the 8 cores, returning
    (out, wall_times_s). Mirrors bass2jax.run_bass_via_pjrt's multi-core
    path but keeps the jitted executable + device-resident inputs for
    repeated timing."""
    global _NC
    import time
    import jax
    import numpy as jnp_np
    from jax.sharding import Mesh, PartitionSpec, NamedSharding
    from jax.experimental.shard_map import shard_map
    from concourse import mybir
    from concourse.bass2jax import (
        _bass_exec_p,
        install_neuronx_cc_hook,
        partition_id_tensor,
    )

    if _NC is None:
        _NC = build_nc()
    nc = _NC
    install_neuronx_cc_hook()
    in_maps = make_host_inputs(x, W)
    n_cores = NCORES

    in_names, out_names, out_avals, zero_outs = [], [], [], []
    partition_name = nc.partition_id_tensor.name if nc.partition_id_tensor else None
    for alloc in nc.m.functions[0].allocations:
        if not isinstance(alloc, mybir.MemoryLocationSet):
            continue
        name = alloc.memorylocations[0].name
        if alloc.kind == "ExternalInput":
            if name != partition_name:
                in_names.append(name)
        elif alloc.kind == "ExternalOutput":
            shape = list(alloc.tensor_shape)
            dt = mybir.dt.np(alloc.dtype)
            out_avals.append(jax.core.ShapedArray(shape, dt))
            out_names.append(name)
            zero_outs.append(np.zeros(shape, dt))
    n_params = len(in_names)
    n_outs = len(out_names)
    all_in_names = list(in_names) + out_names
    if partition_name is not None:
        all_in_names.append(partition_name)

    def _body(*args):
        operands = list(args)
        if partition_name is not None:
            operands.append(partition_id_tensor())
        outs = _bass_exec_p.bind(
            *operands,
            out_avals=tuple(out_avals),
            in_names=tuple(all_in_names),
            out_names=tuple(out_names),
            lowering_input_output_aliases=(),
            sim_require_finite=True,
            sim_require_nnan=True,
            nc=nc,
        )
        return tuple(outs)

    devices = jax.devices()[:n_cores]
    mesh = Mesh(np.asarray(devices), ("core",))
    in_specs = (PartitionSpec("core"),) * (n_params + n_outs)
    out_specs = (PartitionSpec("core"),) * n_outs
    sharded = jax.jit(
        shard_map(_body, mesh=mesh, in_specs=in_specs, out_specs=out_specs,
                  check_rep=False),
        keep_unused=True,
    )
    shard = NamedSharding(mesh, PartitionSpec("core"))
    concat_in = [
        jax.device_put(
            np.concatenate([np.asarray(in_maps[c][nm]) for c in range(n_cores)], 0),
            shard,
        )
        for nm in in_names
    ]
    concat_zeros = [
        jax.device_put(
            np.zeros((n_cores * z.shape[0], *z.shape[1:]), z.dtype), shard
        )
        for z in zero_outs
    ]
    times = []
    out_arrs = None
    for i in range(iters):
        t0 = time.perf_counter()
        out_arrs = sharded(*concat_in, *concat_zeros)
        jax.block_until_ready(out_arrs)
        times.append(time.perf_counter() - t0)
    outs = [
        np.asarray(out_arrs[0]).reshape(n_cores, *out_avals[0].shape)[c]
        for c in range(n_cores)
    ]
    return assemble_out(outs), times



# revision 10
# speedup vs baseline: 63.8587x; 63.8587x over previous
"""Trainium2 Bass kernel for nn_CapsRoutingLayer (capsule dynamic routing).

Sharding: data-parallel over batch. 8 NeuronCores, 8 batch elements each.

Per core:
  phase 1 (build): stream W (host pre-transposed, bf16, contiguous per
    partition) in 16 slabs; for each 16-n chunk run ONE full-width
    128x128 PE matmul (stationary = mask-expanded x, so M=128 output
    rows = 16 n x 8 b) producing x_hat[chunk] in PSUM; accumulate
    s0 = sum_n x_hat with a second PE matmul per chunk; copy PSUM->SBUF
    (bf16) alternating DVE/ACT.
  phase 2 (routing): 3 iterations.
    s-step: z = c (.) x_hat on DVE/Pool, then PE matmuls against a
      fold matrix accumulate s[b,(d,o)] in PSUM (replaces DVE adds).
    a-step: y = v (.) x_hat, fold over d with a pairwise tree split
      across DVE and Pool, accumulate into routing logits.
    softmax over o: ACT exp + DVE tree-sum/recip/mul.

SBUF layout: partition p = 8*n16 + b (n = nb*16 + n16); x_hat free
axis = (nb, d, o) with o innermost so every broadcast operand keeps a
unit-stride innermost dim (DVE 2x mode).

Self-contained: hardcodes all shapes from the problem spec.
  x: (64, 2048, 8) f32;  W: (2048, 32, 16, 8) f32  ->  v: (64, 32, 16) f32
"""

import sys

sys.path.insert(0, "/opt/trn_rl_repo")

import numpy as np
import ml_dtypes

# ---- problem sizes (hardcoded) ----
B_FULL, N, O, D, I = 64, 2048, 32, 16, 8
NCORES = 8
B = B_FULL // NCORES  # 8 batch elements per core
DO = D * O  # 512, on-chip innermost layout is (d, o)
NB = N // 16  # 128 16-n chunks
WSLAB = 8  # chunks per W DMA slab
AGRP = 8  # chunks per a-step group (16 groups)
SGRP = 4  # chunks per s-step group (32 groups)
N_ROUTING = 3

# a-step groups whose mul+f1 run on Pool instead of DVE (engine balance)
POOL_AGRP = frozenset((5, 10, 15))
# s-step groups whose z-mul runs on Pool
POOL_SGRP = frozenset((2, 5, 8, 11, 14, 17, 20, 23, 26, 29))

_NC = {}


def _emit(tc, dram):
    import concourse.bass as bass
    from concourse import mybir

    nc = tc.nc
    BF = mybir.dt.bfloat16
    F32 = mybir.dt.float32
    ACTF = mybir.ActivationFunctionType
    AX = mybir.AxisListType
    ALU = mybir.AluOpType

    wt_d, xt_d, xmask_d, rep8_d, pfold_d, out_d = (
        dram["wt"], dram["xt"], dram["xmask16"], dram["rep8"], dram["pfold"],
        dram["out"],
    )

    from contextlib import ExitStack

    ctx = ExitStack()
    const = ctx.enter_context(tc.tile_pool(name="const", bufs=1))
    persist = ctx.enter_context(tc.tile_pool(name="persist", bufs=1))

    # ---- constants / inputs resident in SBUF ----
    xmask16 = const.tile([128, 128], BF)
    nc.sync.dma_start(out=xmask16[:], in_=xmask_d[:])
    rep8 = const.tile([8, 128], BF)
    nc.sync.dma_start(out=rep8[:], in_=rep8_d[:])
    pfold = const.tile([128, 8], BF)
    nc.sync.dma_start(out=pfold[:], in_=pfold_d[:])
    xt = const.tile([128, NB, B], BF)
    nc.sync.dma_start(out=xt[:], in_=xt_d[:])

    # ---- persistent big tensors ----
    xh = persist.tile([128, NB, DO], BF)  # x_hat, 128KB/partition
    logits = persist.tile([128, NB, O], F32)  # routing logits b
    cw = persist.tile([128, NB, O], BF)  # coupling coefficients c

    with (
        tc.tile_pool(name="s0ps_pool", bufs=1, space="PSUM") as s0pool,
        tc.tile_pool(name="routps", bufs=1, space="PSUM") as rps,
    ):
        s0ps = s0pool.tile([8, DO], F32)

        # ---- phase 1: build x_hat (+ s0 on PE) ----
        with (
            tc.tile_pool(name="wpool", bufs=2) as wpool,
            tc.tile_pool(name="xbpool", bufs=2) as xbpool,
            tc.tile_pool(name="buildps", bufs=2, space="PSUM") as bps,
        ):
            for g in range(NB // WSLAB):
                nb0 = WSLAB * g
                wstep = wpool.tile([128, WSLAB, DO], BF, tag="w", name=f"w_{g}")
                nc.sync.dma_start(
                    out=wstep[:], in_=wt_d[:, nb0 : nb0 + WSLAB, :]
                )
                xb = xbpool.tile([128, WSLAB, 16, B], BF, tag="xb",
                                 name=f"xb_{g}")
                nc.vector.tensor_mul(
                    xb[:],
                    xmask16[:].rearrange("p (n b) -> p n b", n=16)
                    .unsqueeze(1).broadcast_to([128, WSLAB, 16, B]),
                    xt[:, nb0 : nb0 + WSLAB, :]
                    .unsqueeze(2).broadcast_to([128, WSLAB, 16, B]),
                )
                for h in range(WSLAB // 2):
                    ps = bps.tile([128, 2, DO], F32, tag="bps",
                                  name=f"ps_{g}_{h}")
                    for j in range(2):
                        k = 2 * h + j
                        nb = nb0 + k
                        nc.tensor.matmul(
                            ps[:, j, :],
                            xb[:, k, :, :].rearrange("p n b -> p (n b)"),
                            wstep[:, k, :],
                            start=True, stop=True,
                            skip_group_check=True,
                        )
                        nc.tensor.matmul(
                            s0ps[:],
                            xt[:, nb, :],
                            wstep[:, k, :],
                            start=(nb == 0), stop=(nb == NB - 1),
                            skip_group_check=True,
                        )
                    dst = xh[:, nb0 + 2 * h : nb0 + 2 * h + 2, :]
                    if (4 * g + h) % 9 < 5:
                        nc.vector.tensor_copy(dst, ps[:, :, :])
                    else:
                        nc.scalar.copy(dst, ps[:, :, :])

        # ---- phase 2: routing ----
        zpool = ctx.enter_context(tc.tile_pool(name="zpool", bufs=2))
        ypool = ctx.enter_context(tc.tile_pool(name="ypool", bufs=2))
        scratch = ctx.enter_context(tc.tile_pool(name="scratch", bufs=1))
        if True:
            vrep = scratch.tile([128, DO], BF, tag="vrep")
            vsb = scratch.tile([8, DO], BF, tag="vsb")
            # softmax scratch
            zred = scratch.tile([128, NB], F32, tag="zred")
            zrb = scratch.tile([128, NB], BF, tag="zrb")
            # squash smalls
            ssb = scratch.tile([8, DO], F32, tag="ssb")
            ssq = scratch.tile([8, DO], F32, tag="ssq")
            sq1 = scratch.tile([8, 8, O], F32, tag="sq1")
            sq2 = scratch.tile([8, 4, O], F32, tag="sq2")
            sq3 = scratch.tile([8, 2, O], F32, tag="sq3")
            n2 = scratch.tile([8, O], F32, tag="n2")
            nr = scratch.tile([8, O], F32, tag="nr")
            den = scratch.tile([8, O], F32, tag="den")
            fac = scratch.tile([8, O], F32, tag="fac")
            vout = ssq  # ssq is dead by the time vout is written

        def softmax(it):
            # cw = softmax(logits, axis=o); logits bounded, skip max-sub
            nc.scalar.activation(cw[:], logits[:], ACTF.Exp)
            nc.vector.tensor_reduce(
                zred[:], cw[:], axis=AX.X, op=ALU.add
            )
            nc.vector.reciprocal(zred[:], zred[:])
            nc.vector.tensor_copy(zrb[:], zred[:])
            nc.vector.tensor_mul(
                cw[:], cw[:],
                zrb[:].unsqueeze(2).broadcast_to([128, NB, O]),
            )

        def s_step(it):
            # s[b,(d,o)] = sum_n c[b,n,o] * xh[b,n,(d,o)]
            # z products on DVE/Pool; all n-summation on PE via pfold.
            sps = rps.tile([8, DO], F32, tag="sps", name=f"sps_{it}")
            ngrp = NB // SGRP
            for g in range(ngrp):
                nb0 = SGRP * g
                z = zpool.tile([128, SGRP, DO], BF, tag="z",
                               name=f"z_{it}_{g}")
                eng = nc.gpsimd if g in POOL_SGRP else nc.vector
                eng.tensor_mul(
                    z[:].rearrange("p n (d o) -> p n d o", d=D),
                    xh[:, nb0 : nb0 + SGRP, :]
                    .rearrange("p n (d o) -> p n d o", d=D),
                    cw[:, nb0 : nb0 + SGRP, :]
                    .unsqueeze(2).broadcast_to([128, SGRP, D, O]),
                )
                for k in range(SGRP):
                    nc.tensor.matmul(
                        sps[:], pfold[:], z[:, k, :],
                        start=(g == 0 and k == 0),
                        stop=(g == ngrp - 1 and k == SGRP - 1),
                        skip_group_check=True,
                    )
            return sps

        def squash(sps, it, scale):
            last = it == N_ROUTING - 1
            nc.vector.tensor_copy(ssb[:], sps[:])
            nc.vector.tensor_mul(ssq[:], ssb[:], ssb[:])
            sv3 = ssq[:].rearrange("b (d o) -> b d o", d=D)
            nc.vector.tensor_add(sq1[:], sv3[:, 0:8, :], sv3[:, 8:16, :])
            nc.vector.tensor_add(sq2[:], sq1[:, 0:4, :], sq1[:, 4:8, :])
            nc.vector.tensor_add(sq3[:], sq2[:, 0:2, :], sq2[:, 2:4, :])
            nc.vector.tensor_add(n2[:], sq3[:, 0, :], sq3[:, 1, :])
            if scale != 1.0:
                nc.vector.tensor_scalar_mul(n2[:], n2[:], scale * scale)
            nc.scalar.activation(nr[:], n2[:], ACTF.Sqrt)
            nc.vector.tensor_scalar_add(den[:], n2[:], 1.0)
            nc.vector.reciprocal(fac[:], den[:])
            nc.vector.tensor_mul(fac[:], fac[:], nr[:])
            if scale != 1.0:
                nc.vector.tensor_scalar_mul(fac[:], fac[:], scale)
            fb = fac[:].unsqueeze(1).broadcast_to([8, D, O])
            sv = ssb[:].rearrange("b (d o) -> b d o", d=D)
            nc.vector.tensor_mul(vsb[:].rearrange("b (d o) -> b d o", d=D),
                                 sv, fb)
            if last:
                nc.vector.tensor_mul(
                    vout[:].rearrange("b (d o) -> b d o", d=D), sv, fb
                )
                nc.sync.dma_start(out=out_d[:], in_=vout[:])

        def a_step(it):
            # logits[b,n,o] (+)= sum_d v[b,(d,o)] * xh[b,n,(d,o)]
            first = it == 0
            vps = rps.tile([128, DO], F32, tag="vps", name=f"vps_{it}")
            nc.tensor.matmul(vps[:], rep8[:], vsb[:], start=True, stop=True)
            nc.vector.tensor_copy(vrep[:], vps[:])
            vb = (
                vrep[:].rearrange("p (d o) -> p d o", d=D)
                .unsqueeze(1).broadcast_to([128, AGRP, D, O])
            )
            for g in range(NB // AGRP):
                nb0 = AGRP * g
                eng = nc.gpsimd if g in POOL_AGRP else nc.vector
                y = ypool.tile([128, AGRP, D, O], BF, tag="y",
                               name=f"y_{it}_{g}")
                f1 = ypool.tile([128, AGRP, 8, O], BF, tag="f1",
                                name=f"f1_{it}_{g}")
                f2 = ypool.tile([128, AGRP, 4, O], BF, tag="f2",
                                name=f"f2_{it}_{g}")
                f3 = scratch.tile([128, AGRP, 2, O], BF, tag="f3")
                f4 = scratch.tile([128, AGRP, O], BF, tag="f4")
                eng.tensor_mul(
                    y[:],
                    xh[:, nb0 : nb0 + AGRP, :]
                    .rearrange("p n (d o) -> p n d o", d=D),
                    vb,
                )
                eng.tensor_add(f1[:], y[:, :, 0:8, :], y[:, :, 8:16, :])
                nc.gpsimd.tensor_add(f2[:], f1[:, :, 0:4, :], f1[:, :, 4:8, :])
                nc.gpsimd.tensor_add(f3[:], f2[:, :, 0:2, :], f2[:, :, 2:4, :])
                nc.gpsimd.tensor_add(f4[:], f3[:, :, 0, :], f3[:, :, 1, :])
                lsl = logits[:, nb0 : nb0 + AGRP, :]
                if first:
                    nc.gpsimd.tensor_copy(lsl, f4[:])
                else:
                    nc.gpsimd.tensor_add(lsl, lsl, f4[:])

        for it in range(N_ROUTING):
            if it == 0:
                squash(s0ps, it, scale=1.0 / O)
            else:
                softmax(it)
                sps = s_step(it)
                squash(sps, it, scale=1.0)
            if it < N_ROUTING - 1:
                a_step(it)

    ctx.close()


def build_nc(reps=1):
    import concourse.bass as bass
    import concourse.tile as tile
    from concourse import bacc, mybir

    BF = mybir.dt.bfloat16
    F32 = mybir.dt.float32
    nc = bacc.Bacc(
        "TRN2",
        target_bir_lowering=False,
        debug=False,
        enable_asserts=False,
        num_devices=NCORES,
    )
    dram = {
        "wt": nc.dram_tensor("wt", [128, NB, DO], BF, kind="ExternalInput").ap(),
        "xt": nc.dram_tensor("xt", [128, NB, B], BF, kind="ExternalInput").ap(),
        "xmask16": nc.dram_tensor(
            "xmask16", [128, 128], BF, kind="ExternalInput"
        ).ap(),
        "rep8": nc.dram_tensor("rep8", [8, 128], BF, kind="ExternalInput").ap(),
        "pfold": nc.dram_tensor("pfold", [128, 8], BF, kind="ExternalInput").ap(),
        "out": nc.dram_tensor("out", [B, DO], F32, kind="ExternalOutput").ap(),
    }
    with tile.TileContext(nc) as tc:
        for _ in range(reps):
            _emit(tc, dram)
    nc.compile()
    return nc


def make_host_inputs(x, W):
    """Host-side layout prep. Returns per-core in_maps."""
    bf = ml_dtypes.bfloat16
    x = np.asarray(x, np.float32)
    W = np.asarray(W, np.float32)
    # W (N, O, D, I) -> wt[p=(m,i), nb, (d,o)] with n = nb*16 + m
    wt = (
        np.ascontiguousarray(
            W.reshape(NB, 16, O, D, I).transpose(1, 4, 0, 3, 2)
        )
        .reshape(128, NB, DO)
        .astype(bf)
    )
    p = np.arange(128)
    xmask16 = ((p[:, None] // 8) == (np.arange(128)[None, :] // 8)).astype(bf)
    rep8 = (np.arange(8)[:, None] == (np.arange(128)[None, :] % 8)).astype(bf)
    pfold = ((np.arange(128)[:, None] % 8) == np.arange(8)[None, :]).astype(bf)
    in_maps = []
    for c in range(NCORES):
        xc = x[B * c : B * c + B]  # (B, N, I)
        # xt[p=(m,i), nb, b] = x[b, nb*16+m, i]
        xt = (
            np.ascontiguousarray(
                xc.reshape(B, NB, 16, I).transpose(2, 3, 1, 0)
            )
            .reshape(128, NB, B)
            .astype(bf)
        )
        in_maps.append(
            {"wt": wt, "xt": xt, "xmask16": xmask16, "rep8": rep8,
             "pfold": pfold}
        )
    return in_maps


def assemble_out(core_outs):
    """core_outs[k]: (B, DO) f32 in (d, o) layout -> (64, O, D) f32."""
    outs = [
        np.asarray(o, np.float32).reshape(B, D, O).transpose(0, 2, 1)
        for o in core_outs
    ]
    return np.ascontiguousarray(np.concatenate(outs, axis=0))


def run(x, W, trace=False, reps=1):
    """Build (cached), execute on 8 cores, return (out, exec_time_ns)."""
    from concourse.bass_utils import run_bass_kernel_spmd

    if reps not in _NC:
        _NC[reps] = build_nc(reps)
    in_maps = make_host_inputs(x, W)
    res = run_bass_kernel_spmd(
        _NC[reps], in_maps, core_ids=list(range(NCORES)), trace=trace
    )
    out = assemble_out([res.results[k]["out"] for k in range(NCORES)])
    return out, res.exec_time_ns


def kernel(x, W):
    import time

    for attempt in range(3):
        try:
            out, _ = run(x, W, trace=False)
            return out
        except Exception:
            if attempt == 2:
                raise
            time.sleep(2.0)


def _make_runner(nc, in_maps):
    """Build a jitted 8-core runner for a compiled NEFF with
    device-resident inputs. Returns (call, extract_out)."""
    import jax
    from jax.sharding import Mesh, PartitionSpec, NamedSharding
    from jax.experimental.shard_map import shard_map
    from concourse import mybir
    from concourse.bass2jax import (
        _bass_exec_p,
        install_neuronx_cc_hook,
        partition_id_tensor,
    )

    install_neuronx_cc_hook()
    n_cores = NCORES

    in_names, out_names, out_avals, zero_outs = [], [], [], []
    partition_name = nc.partition_id_tensor.name if nc.partition_id_tensor else None
    for alloc in nc.m.functions[0].allocations:
        if not isinstance(alloc, mybir.MemoryLocationSet):
            continue
        name = alloc.memorylocations[0].name
        if alloc.kind == "ExternalInput":
            if name != partition_name:
                in_names.append(name)
        elif alloc.kind == "ExternalOutput":
            shape = list(alloc.tensor_shape)
            dt = mybir.dt.np(alloc.dtype)
            out_avals.append(jax.core.ShapedArray(shape, dt))
            out_names.append(name)
            zero_outs.append(np.zeros(shape, dt))
    n_params = len(in_names)
    n_outs = len(out_names)
    all_in_names = list(in_names) + out_names
    if partition_name is not None:
        all_in_names.append(partition_name)

    def _body(*args):
        operands = list(args)
        if partition_name is not None:
            operands.append(partition_id_tensor())
        outs = _bass_exec_p.bind(
            *operands,
            out_avals=tuple(out_avals),
            in_names=tuple(all_in_names),
            out_names=tuple(out_names),
            lowering_input_output_aliases=(),
            sim_require_finite=True,
            sim_require_nnan=True,
            nc=nc,
        )
        return tuple(outs)

    devices = jax.devices()[:n_cores]
    mesh = Mesh(np.asarray(devices), ("core",))
    in_specs = (PartitionSpec("core"),) * (n_params + n_outs)
    out_specs = (PartitionSpec("core"),) * n_outs
    sharded = jax.jit(
        shard_map(_body, mesh=mesh, in_specs=in_specs, out_specs=out_specs,
                  check_rep=False),
        keep_unused=True,
    )
    shard = NamedSharding(mesh, PartitionSpec("core"))
    concat_in = [
        jax.device_put(
            np.concatenate([np.asarray(in_maps[c][nm]) for c in range(n_cores)], 0),
            shard,
        )
        for nm in in_names
    ]
    concat_zeros = [
        jax.device_put(
            np.zeros((n_cores * z.shape[0], *z.shape[1:]), z.dtype), shard
        )
        for z in zero_outs
    ]

    def call():
        out_arrs = sharded(*concat_in, *concat_zeros)
        jax.block_until_ready(out_arrs)
        return out_arrs

    def extract_out(out_arrs):
        outs = [
            np.asarray(out_arrs[0]).reshape(n_cores, *out_avals[0].shape)[c]
            for c in range(n_cores)
        ]
        return assemble_out(outs)

    return call, extract_out


def bench_hw(x, W, iters=30, reps=1):
    """Repeat-execute the kernel NEFF on the 8 cores, returning
    (out, wall_times_s)."""
    import time

    if reps not in _NC:
        _NC[reps] = build_nc(reps)
    in_maps = make_host_inputs(x, W)
    call, extract_out = _make_runner(_NC[reps], in_maps)
    times = []
    out_arrs = None
    for i in range(iters):
        t0 = time.perf_counter()
        out_arrs = call()
        times.append(time.perf_counter() - t0)
    return extract_out(out_arrs), times


def bench_interleaved(x, W, iters=60, reps_list=(1, 9)):
    """Time several NEFF variants (differing only in body repetitions)
    with strictly interleaved dispatch, so slow drift in the tunnel
    dispatch overhead affects every variant equally. Returns
    (outs[reps], times[reps])."""
    import time

    for r in reps_list:
        if r not in _NC:
            _NC[r] = build_nc(r)
    in_maps = make_host_inputs(x, W)
    runners = {r: _make_runner(_NC[r], in_maps) for r in reps_list}
    # warmup
    last = {}
    for r in reps_list:
        for _ in range(2):
            last[r] = runners[r][0]()
    times = {r: [] for r in reps_list}
    for i in range(iters):
        for r in reps_list:
            t0 = time.perf_counter()
            last[r] = runners[r][0]()
            times[r].append(time.perf_counter() - t0)
    outs = {r: runners[r][1](last[r]) for r in reps_list}
    return outs, times
